# revision 83
# baseline (speedup 1.0000x reference)
"""Trainium2 Bass kernel for nn_Block_19069654794616 (dense transformer block).

B=2, S=2048, D=1600, 25 heads x 64, causal attention, 4x MLP (tanh-gelu),
pre-LN with residuals. fp32 in/out, bf16 matmul operands internally.

Distribution (8 NeuronCores, token-parallel):
  Core j owns 512 tokens: chunk A = seq0[256j:256j+256], chunk B =
  seq1[256(7-j):256(8-j)] (mirrored pairing balances the causal triangle).
  - LN1 + QKV per-core on its own tokens (LN gains folded into the QKV
    weights host-side; q pre-scaled by c^-0.5). All weights bf16.
  - k^T / v shards (bf16, v padded with a per-head ones column for the
    softmax denominator) AllGather'd across the 8 cores.
  - Attention: all 25 heads for the 2 query chunks over the full 2048-token
    prefix. Causal/validity masking is a multiplicative 0/1 bf16 mask applied
    on the DVE after exp. AV accumulates in PSUM across all 8 key
    super-chunks (one bank per head pair).
  - proj / LN2 / MLP token-local; outputs concatenated host-side.
"""

import numpy as np

import concourse.bass as bass
import concourse.mybir as mybir
import concourse.tile as tile
from concourse import bacc
from concourse.bass_utils import run_bass_kernel_spmd
from concourse.masks import make_identity

f32 = mybir.dt.float32
bf16 = mybir.dt.bfloat16

N_CORES = 8
B, S, D = 2, 2048, 1600
H, C = 25, 64
D3, D4 = 3 * D, 4 * D
TOK = 512          # tokens per core
CH = 256           # query chunk (2 per core)
LC = 128           # key sub-block
NHP = 13           # head-pair tiles (12 pairs + head 24)
NSC = 8            # key super-chunks (256 tok) per sequence
EPS = 1e-5
VW = 65            # per-head v width (64 + ones column)

# D contraction chunks: 12x128 + 1x64
DCH = [(t * 128, 128) for t in range(12)] + [(1536, 64)]
# output-column tiles of 400 for D-sized outputs (proj / mlp out)
NJ = [(j * 400, 400) for j in range(4)]
# v-proj blocks: 5 heads (320 cols) each
VB = [(b * 5, 5) for b in range(5)]
# mlp f-tile groups (of 50 x 128-col tiles); even pair counts
GRP = [14, 12, 12, 12]

KREG = NHP * 128 * TOK          # bf16 elems in the k^T region of a shard
VREG = 4 * 128 * (H * VW)      # bf16 elems in the v region (ones-padded)
SHARD = KREG + VREG
SHARDP = SHARD                  # slot pitch in kv_all (collective output
                                # must be contiguous per the BIR verifier)
VROW = H * VW                   # 1625


def _build():
    nc = bacc.Bacc(
        "TRN2",
        target_bir_lowering=False,
        debug=False,
        enable_asserts=True,
        num_devices=N_CORES,
    )
    x_in = nc.dram_tensor("x", [TOK, D], f32, kind="ExternalInput").ap()
    wqkv = nc.dram_tensor("wqkv", [D, D3], bf16, kind="ExternalInput").ap()
    bqkv = nc.dram_tensor("bqkv", [D3], f32, kind="ExternalInput").ap()
    wproj = nc.dram_tensor("wproj", [D, D], bf16, kind="ExternalInput").ap()
    bproj = nc.dram_tensor("bproj", [D], f32, kind="ExternalInput").ap()
    wfc = nc.dram_tensor("wfc", [D, D4], bf16, kind="ExternalInput").ap()
    bfc = nc.dram_tensor("bfc", [D4], f32, kind="ExternalInput").ap()
    wout = nc.dram_tensor("wout", [D4, D], bf16, kind="ExternalInput").ap()
    bout = nc.dram_tensor("bout", [D], f32, kind="ExternalInput").ap()
    masks = nc.dram_tensor("masks", [2, NSC, 2, LC, CH], bf16,
                           kind="ExternalInput").ap()
    out = nc.dram_tensor("out", [TOK, D], f32, kind="ExternalOutput").ap()

    shard = nc.dram_tensor("shard", [1, SHARD], bf16, kind="Internal").ap()
    stg_dram = nc.dram_tensor("stg_dram", [NHP, 2, 2, VW, CH], bf16,
                              kind="Internal").ap()
    kv_all = nc.dram_tensor(
        "kv_all", [N_CORES, SHARDP], bf16, kind="Internal", addr_space="Shared"
    ).ap()

    with tile.TileContext(nc, pool_alloc_mode="queue") as tc:
        _emit(tc, nc, x_in, wqkv, bqkv, wproj, bproj, wfc, bfc, wout, bout,
              masks, out, shard, kv_all, stg_dram)
    nc.compile()
    return nc


def _emit(tc, nc, x_in, wqkv, bqkv, wproj, bproj, wfc, bfc, wout, bout,
          masks, out, shard, kv_all, stg_dram):
    sync, vec, act, gp, te = nc.sync, nc.vector, nc.scalar, nc.gpsimd, nc.tensor
    AluOp = mybir.AluOpType
    Act = mybir.ActivationFunctionType

    # ---------------- whole-kernel pools ----------------
    psB = tc.alloc_tile_pool(name="psB", bufs=4, space="PSUM")  # qkv-era
    persist = tc.alloc_tile_pool(name="persist", bufs=1)

    identb = persist.tile([128, 128], bf16, name="identb")
    make_identity(nc, identb)
    eps_t = persist.tile([128, 1], f32, name="eps_t")
    vec.memset(eps_t, EPS)

    # per-partition bias columns for k / q / fc (out-channel on partitions)
    bk_col = persist.tile([128, NHP], f32, name="bk_col")
    bq_col = persist.tile([128, NHP], f32, name="bq_col")
    bfc_col = persist.tile([128, 50], f32, name="bfc_col")
    gp.dma_start(out=bq_col, in_=bass.AP(
        tensor=bqkv.tensor, offset=0, ap=[[1, 128], [128, NHP]]))
    gp.dma_start(out=bk_col, in_=bass.AP(
        tensor=bqkv.tensor, offset=D, ap=[[1, 128], [128, NHP]]))
    gp.dma_start(out=bfc_col, in_=bass.AP(
        tensor=bfc.tensor, offset=0, ap=[[1, 128], [128, 50]]))
    # replicated (per-free) bias rows
    bv_rep = persist.tile([128, D], f32, name="bv_rep")
    bproj_rep = persist.tile([128, D], f32, name="bproj_rep")
    bout_rep = persist.tile([128, D], f32, name="bout_rep")
    act.dma_start(out=bv_rep, in_=bass.AP(
        tensor=bqkv.tensor, offset=2 * D, ap=[[0, 128], [1, D]]))
    act.dma_start(out=bproj_rep, in_=bass.AP(
        tensor=bproj.tensor, offset=0, ap=[[0, 128], [1, D]]))
    act.dma_start(out=bout_rep, in_=bass.AP(
        tensor=bout.tensor, offset=0, ap=[[0, 128], [1, D]]))

    def load_w_cols(pool, src, ncols, col0, cw, name, tag, bufs=2, eng=None):
        """(1600, cw) weight slice -> (128, 13, cw) bf16 tile (2 DMAs)."""
        w = pool.tile([128, NHP, cw], bf16, name=name, tag=tag, bufs=bufs)
        src_t = src.tensor
        e = eng or sync
        e.dma_start(out=w[:, 0:12, :], in_=bass.AP(
            tensor=src_t, offset=col0,
            ap=[[ncols, 128], [128 * ncols, 12], [1, cw]]))
        e.dma_start(out=w[:64, 12:13, :], in_=bass.AP(
            tensor=src_t, offset=1536 * ncols + col0,
            ap=[[ncols, 64], [128 * ncols, 1], [1, cw]]))
        return w

    def ln_transpose(get_src, dst_tiles, pool, label, pspool):
        """get_src(tt) -> (128, D) fp32 SBUF tile; LN + transpose into
        13 (128, TOK) bf16 dst tiles (columns tt*128..)."""
        for tt in range(4):
            xt = get_src(tt)
            stats = pool.tile([128, 4, 6], f32, name=f"{label}st{tt}",
                              tag=f"{label}st")
            xg = xt.rearrange("p (g d) -> p g d", g=4)
            for g in range(4):
                vec.bn_stats(out=stats[:, g, :], in_=xg[:, g, :])
            mv = pool.tile([128, 2], f32, name=f"{label}mv{tt}", tag=f"{label}mv")
            vec.bn_aggr(out=mv, in_=stats)
            rstd = pool.tile([128, 1], f32, name=f"{label}rs{tt}",
                             tag=f"{label}rs")
            act.activation(out=rstd, in_=mv[:, 1:2], func=Act.Sqrt, bias=eps_t)
            vec.reciprocal(out=rstd, in_=rstd)
            xc = pool.tile([128, D], bf16, name=f"{label}xc{tt}",
                           tag=f"{label}xc")
            vec.tensor_scalar(out=xc, in0=xt, scalar1=mv[:, 0:1], scalar2=rstd,
                              op0=AluOp.subtract, op1=AluOp.mult)
            for t, (d0, dp) in enumerate(DCH):
                tp = pspool.tile([128, 128], bf16, name=f"{label}tp",
                                 tag="lntp", bufs=2)
                te.transpose(tp[:dp, :], xc[:, d0:d0 + dp], identb)
                act.activation(out=dst_tiles[t][:dp, tt * 128:(tt + 1) * 128],
                               in_=tp[:dp, :], func=Act.Copy)

    # ======== S1: LN1 -> xcT; k,v; shard; AllGather; q ========
    pool_y = tc.alloc_tile_pool(name="pool_y", bufs=1)
    y = [pool_y.tile([128, D], f32, name=f"y{tt}", tag=f"y{tt}")
         for tt in range(4)]
    pool_qT = tc.alloc_tile_pool(name="pool_qT", bufs=1)
    qT = [pool_qT.tile([128, TOK], bf16, name=f"qT{t}", tag=f"qT{t}")
          for t in range(NHP)]
    pool_s1 = tc.alloc_tile_pool(name="pool_s1", bufs=1)
    kT = [pool_s1.tile([128, TOK], bf16, name=f"kT{t}", tag=f"kT{t}")
          for t in range(NHP)]
    vown = [pool_s1.tile([128, H, VW], bf16, name=f"vown{tt}", tag=f"vown{tt}")
            for tt in range(4)]
    pool_xc = tc.alloc_tile_pool(name="pool_xc", bufs=1)
    xcT = [pool_xc.tile([128, TOK], bf16, name=f"xcT{t}", tag=f"xcT{t}")
           for t in range(NHP)]
    for tt in range(4):
        # ones columns for the softmax denominator (never overwritten)
        gp.memset(vown[tt][:, :, 64:65], 1.0)

    pool_ln = tc.alloc_tile_pool(name="pool_ln", bufs=2)

    def ln1_src(tt):
        xt = pool_ln.tile([128, D], f32, name=f"ln1x{tt}", tag="ln1x")
        for g in range(4):
            sync.dma_start(out=xt[:, g * 400:(g + 1) * 400],
                           in_=x_in[tt * 128:(tt + 1) * 128,
                                    g * 400:(g + 1) * 400])
        return xt

    ln_transpose(ln1_src, xcT, pool_ln, "ln1", pspool=psB)
    pool_ln.release()

    pool_w1 = tc.alloc_tile_pool(name="pool_w1", bufs=1)
    kreg = shard[0, 0:KREG].rearrange("(t p n) -> t p n", t=NHP, p=128)
    vreg = shard[0, KREG:SHARD].rearrange("(t p n) -> t p n", t=4, p=128)

    def qk_proj(col_base, bias_col, dst, to_shard):
        for tp in range(7):
            nt = 2 if tp < 6 else 1
            w = load_w_cols(pool_w1, wqkv, D3, col_base + tp * 256, nt * 128,
                            f"w{col_base}_{tp}", tag="wbig", bufs=3)
            for ti in range(nt):
                t = 2 * tp + ti
                ps = psB.tile([128, TOK], f32, name="qkps", tag="ps1", bufs=2)
                for ci, (d0, dp) in enumerate(DCH):
                    te.matmul(ps, lhsT=w[:dp, ci, ti * 128:(ti + 1) * 128],
                              rhs=xcT[ci][:dp, :],
                              start=(ci == 0), stop=(ci == len(DCH) - 1))
                act.activation(out=dst[t], in_=ps, func=Act.Identity,
                               bias=bias_col[:, t:t + 1])
                if to_shard:
                    act.dma_start(out=kreg[t], in_=dst[t])

    qk_proj(D, bk_col, kT, True)          # k^T -> shard

    # v (natural layout, ones-padded per head) + bias
    vps = {}
    for b0, nb in VB:
        wv = load_w_cols(pool_w1, wqkv, D3, 2 * D + b0 * 64, 320,
                         f"wv{b0}", tag="wv", bufs=2)
        for ci, (d0, dp) in enumerate(DCH):
            for tt in range(4):
                if ci == 0:
                    vps[tt] = psB.tile([128, 320], f32, name=f"vps{tt}",
                                       tag="vps")
                te.matmul(vps[tt], lhsT=xcT[ci][:dp, tt * 128:(tt + 1) * 128],
                          rhs=wv[:dp, ci, :], start=(ci == 0),
                          stop=(ci == len(DCH) - 1))
        for tt in range(4):
            vec.tensor_tensor(
                out=vown[tt][:, b0:b0 + nb, 0:64],
                in0=vps[tt].rearrange("p (h c) -> p h c", h=nb),
                in1=bv_rep[:, b0 * 64:(b0 + nb) * 64].rearrange(
                    "p (h c) -> p h c", h=nb),
                op=AluOp.add)
    for tt in range(4):
        act.dma_start(out=vreg[tt], in_=vown[tt])

    gp.collective_compute(
        "AllGather", mybir.AluOpType.bypass,
        replica_groups=[list(range(N_CORES))],
        ins=[shard], outs=[kv_all[:, 0:SHARD]],
    )

    qk_proj(0, bq_col, qT, False)          # q^T (pre-scaled host-side)

    pool_w1.release()
    pool_xc.release()

    # ======== attention ========
    psB.release()
    psSt = tc.alloc_tile_pool(name="psSt", bufs=2, space="PSUM")   # 4 banks
    psAv = tc.alloc_tile_pool(name="psAv", bufs=2, space="PSUM")   # 2 banks
    psP = tc.alloc_tile_pool(name="psP", bufs=2, space="PSUM")     # 2 banks

    # ---- diagonal pre-pass: own-core k/v (still in SBUF), runs inside the
    # AllGather window; partial AV + denominators staged to DRAM. ----
    pool_pre = tc.alloc_tile_pool(name="pool_pre", bufs=2)
    mdiag = []
    for c in range(2):
        m = pool_pre.tile([128, 2, 2, CH], bf16, name=f"md{c}", tag=f"md{c}",
                          bufs=1)
        for hh in range(2):
            sync.dma_start(out=m[:, hh, :, :], in_=bass.AP(
                tensor=masks.tensor, offset=(c * NSC + 7) * 2 * LC * CH,
                ap=[[CH, 128], [LC * CH, 2], [1, CH]]))
        mdiag.append(m.rearrange("p a b q -> p (a b) q"))
    for hp in range(NHP):
        nh = 1 if hp == 12 else 2
        for c in range(2):
            toff = c * CH
            avd = psAv.tile([VW, 2, CH], f32, name="avd", tag="av")
            st = psSt.tile([128, 4, CH], f32, name="std", tag="st")
            for hh in range(nh):
                p0 = hh * 64
                for lc in range(2):
                    te.matmul(st[:, hh * 2 + lc, :],
                              lhsT=kT[hp][p0:p0 + 64,
                                          toff + lc * LC:toff + (lc + 1) * LC],
                              rhs=qT[hp][p0:p0 + 64, toff:toff + CH],
                              start=(lc == 0), stop=(lc == 1))
            ptm = pool_pre.tile([128, 4, CH], bf16, name="ptmd", tag="ptmd")
            act.activation(out=ptm[:, 0:2 * nh, :], in_=st[:, 0:2 * nh, :],
                           func=Act.Exp)
            vec.tensor_tensor(out=ptm[:, 0:2 * nh, :],
                              in0=ptm[:, 0:2 * nh, :],
                              in1=mdiag[c][:, 0:2 * nh, :], op=AluOp.mult)
            for hh in range(nh):
                h = 2 * hp + hh
                for lc in range(2):
                    te.matmul(avd[:, hh, :],
                              lhsT=vown[2 * c + lc][:, h, :],
                              rhs=ptm[:, hh * 2 + lc, :],
                              start=(hh == 0 and lc == 0),
                              stop=(hh == nh - 1 and lc == 1))
            sb = pool_pre.tile([VW, 2, CH], bf16, name="sb", tag="sb")
            vec.tensor_copy(out=sb[:, 0:nh, :], in_=avd[:, 0:nh, :])
            act.dma_start(out=stg_dram[hp, c], in_=sb)
    pool_pre.release()
    pool_s1.release()

    pool_at = tc.alloc_tile_pool(name="pool_at", bufs=1)
    attn_T = [pool_at.tile([128, TOK], bf16, name=f"attnT{t}", tag=f"attnT{t}")
              for t in range(NHP)]
    pool_msk = tc.alloc_tile_pool(name="pool_msk", bufs=1)
    pool_kv = tc.alloc_tile_pool(name="pool_kv", bufs=4)
    pool_ptm = tc.alloc_tile_pool(name="pool_ptm", bufs=4)
    pool_nrm = tc.alloc_tile_pool(name="pool_nrm", bufs=4)
    pool_pj = tc.alloc_tile_pool(name="pool_pj", bufs=2)

    mkk = []
    for c in range(2):
        row = []
        for s in range(NSC - 1):
            m = pool_msk.tile([128, 2, 2, CH], bf16, name=f"mk{c}_{s}",
                              tag=f"mk{c}_{s}")
            for hh in range(2):
                sync.dma_start(out=m[:, hh, :, :], in_=bass.AP(
                    tensor=masks.tensor, offset=(c * NSC + s) * 2 * LC * CH,
                    ap=[[CH, 128], [LC * CH, 2], [1, CH]]))
            row.append(m.rearrange("p a b q -> p (a b) q"))
        mkk.append(row)

    for c in range(2):
        toff = c * CH
        mk = mkk[c]
        for hp in range(NHP):
            nh = 1 if hp == 12 else 2
            ktg = pool_kv.tile([128, NSC, CH], bf16, name="ktg", tag="ktg")
            gp.dma_start(out=ktg, in_=bass.AP(
                tensor=kv_all.tensor, offset=hp * 128 * TOK + toff,
                ap=[[TOK, 128], [SHARDP, NSC], [1, CH]]))
            vtg = pool_kv.tile([128, NSC, 2, 2 * VW], bf16, name="vtg",
                               tag="vtg")
            for lc in range(2):
                gp.dma_start(out=vtg[:, :, lc, 0:nh * VW], in_=bass.AP(
                    tensor=kv_all.tensor,
                    offset=KREG + (2 * c + lc) * 128 * VROW + hp * 2 * VW,
                    ap=[[VROW, 128], [SHARDP, NSC], [1, nh * VW]]))
            stgi = pool_kv.tile([VW, 2, CH], bf16, name="stgi", tag="stgi",
                                bufs=3)
            gp.dma_start(out=stgi, in_=stg_dram[hp, c])
            av = psAv.tile([VW, 2, CH], f32, name="av", tag="av")
            # seed the accumulator with the pre-computed diagonal partial
            for hh in range(nh):
                te.matmul(av[:, hh, :], lhsT=identb[0:VW, 0:VW],
                          rhs=stgi[:, hh, :],
                          start=(hh == 0), stop=False)
            for s in range(NSC - 1):
                r = s if c == 0 else 7 - s
                st = psSt.tile([128, 4, CH], f32, name="st", tag="st")
                for hh in range(nh):
                    p0 = hh * 64
                    for lc in range(2):
                        te.matmul(st[:, hh * 2 + lc, :],
                                  lhsT=ktg[p0:p0 + 64, r, lc * LC:(lc + 1) * LC],
                                  rhs=qT[hp][p0:p0 + 64, toff:toff + CH],
                                  start=(lc == 0), stop=(lc == 1))
                ptm = pool_ptm.tile([128, 4, CH], bf16, name="ptm", tag="ptm")
                act.activation(out=ptm[:, 0:2 * nh, :], in_=st[:, 0:2 * nh, :],
                               func=Act.Exp)
                vec.tensor_tensor(out=ptm[:, 0:2 * nh, :],
                                  in0=ptm[:, 0:2 * nh, :],
                                  in1=mk[s][:, 0:2 * nh, :], op=AluOp.mult)
                for hh in range(nh):
                    for lc in range(2):
                        te.matmul(av[:, hh, :],
                                  lhsT=vtg[:, r, lc, hh * VW:(hh + 1) * VW],
                                  rhs=ptm[:, hh * 2 + lc, :],
                                  start=False,
                                  stop=(s == NSC - 2 and hh == nh - 1
                                        and lc == 1))
            for hh in range(nh):
                rcp = pool_nrm.tile([1, CH], f32, name="rcp", tag="rcp")
                vec.reciprocal(out=rcp, in_=av[64:65, hh, :])
                rcpb = pool_nrm.tile([64, CH], f32, name="rcpb", tag="rcpb")
                gp.partition_broadcast(rcpb, rcp)
                vec.tensor_tensor(
                    out=attn_T[hp][hh * 64:(hh + 1) * 64, toff:toff + CH],
                    in0=av[0:64, hh, :], in1=rcpb, op=AluOp.mult)

        # ---- proj + residual for this chunk's 2 token tiles ----
        pps = {}
        for j0, (c0, cw) in enumerate(NJ):
            wp = pool_pj.tile([128, NHP, cw], bf16, name=f"wp{j0}", tag="wp")
            sync.dma_start(out=wp[:, 0:12, :], in_=bass.AP(
                tensor=wproj.tensor, offset=c0,
                ap=[[D, 128], [128 * D, 12], [1, cw]]))
            sync.dma_start(out=wp[:64, 12:13, :], in_=bass.AP(
                tensor=wproj.tensor, offset=1536 * D + c0,
                ap=[[D, 64], [128 * D, 1], [1, cw]]))
            for tt in (2 * c, 2 * c + 1):
                pps[tt] = psP.tile([128, cw], f32, name=f"pps{tt}", tag="pps")
                for ci, (d0, dp) in enumerate(DCH):
                    te.matmul(pps[tt],
                              lhsT=attn_T[ci][:dp, tt * 128:(tt + 1) * 128],
                              rhs=wp[:dp, ci, :], start=(ci == 0),
                              stop=(ci == len(DCH) - 1))
                xr = pool_pj.tile([128, cw], f32, name=f"xr{tt}", tag="xr",
                                  bufs=3)
                sync.dma_start(out=xr,
                               in_=x_in[tt * 128:(tt + 1) * 128, c0:c0 + cw])
                vec.tensor_tensor(out=y[tt][:, c0:c0 + cw], in0=pps[tt],
                                  in1=xr, op=AluOp.add)
                vec.tensor_tensor(out=y[tt][:, c0:c0 + cw],
                                  in0=y[tt][:, c0:c0 + cw],
                                  in1=bproj_rep[:, c0:c0 + cw], op=AluOp.add)

    pool_pj.release()
    pool_nrm.release()
    pool_ptm.release()
    pool_kv.release()
    pool_msk.release()
    pool_at.release()
    pool_qT.release()
    psP.release()
    psAv.release()
    psSt.release()

    # ======== LN2 -> ycT; MLP; out ========
    psT = tc.alloc_tile_pool(name="psT", bufs=2, space="PSUM")
    pool_s4 = tc.alloc_tile_pool(name="pool_s4", bufs=1)
    ycT = [pool_s4.tile([128, TOK], bf16, name=f"ycT{t}", tag=f"ycT{t}")
           for t in range(NHP)]
    pool_ln2 = tc.alloc_tile_pool(name="pool_ln2", bufs=2)
    ln_transpose(lambda tt: y[tt], ycT, pool_ln2, "ln2", pspool=psT)
    pool_ln2.release()
    psT.release()
    psD = tc.alloc_tile_pool(name="psD", bufs=3, space="PSUM")
    psC = tc.alloc_tile_pool(name="psC", bufs=4, space="PSUM")

    pool_h = tc.alloc_tile_pool(name="pool_h", bufs=2)
    pool_w2 = tc.alloc_tile_pool(name="pool_w2", bufs=1)

    ops = {}
    f_base = 0
    for ng in GRP:
        hT = [pool_h.tile([128, TOK], bf16, name=f"hT{f_base}_{fi}",
                          tag=f"hT{fi}") for fi in range(ng)]
        for fp in range(ng // 2):
            wf = load_w_cols(pool_w2, wfc, D4, (f_base + 2 * fp) * 128, 256,
                             f"wf{f_base}_{fp}", tag="wf", bufs=4)
            for ti in range(2):
                fi = 2 * fp + ti
                f = f_base + fi
                ps = psD.tile([128, TOK], f32, name="hps", tag="hps")
                for ci, (d0, dp) in enumerate(DCH):
                    te.matmul(ps, lhsT=wf[:dp, ci, ti * 128:(ti + 1) * 128],
                              rhs=ycT[ci][:dp, :],
                              start=(ci == 0), stop=(ci == len(DCH) - 1))
                act.activation(out=hT[fi], in_=ps, func=Act.Gelu_apprx_tanh,
                               bias=bfc_col[:, f:f + 1], scale=1.0)
        for j0, (c0, cw) in enumerate(NJ):
            wo = pool_w2.tile([128, ng, cw], bf16, name=f"wo{f_base}_{j0}",
                              tag="wo", bufs=2)
            sync.dma_start(out=wo, in_=bass.AP(
                tensor=wout.tensor, offset=f_base * 128 * D + c0,
                ap=[[D, 128], [128 * D, ng], [1, cw]]))
            for tt in range(4):
                ops[tt] = psC.tile([128, cw], f32, name=f"ops{tt}", tag="ops")
                for fi in range(ng):
                    te.matmul(ops[tt], lhsT=hT[fi][:, tt * 128:(tt + 1) * 128],
                              rhs=wo[:, fi, :], start=(fi == 0),
                              stop=(fi == ng - 1))
            last = (f_base + ng == 50)
            for tt in range(4):
                vec.tensor_tensor(out=y[tt][:, c0:c0 + cw],
                                  in0=y[tt][:, c0:c0 + cw], in1=ops[tt],
                                  op=AluOp.add)
                if last:
                    vec.tensor_tensor(out=y[tt][:, c0:c0 + cw],
                                      in0=y[tt][:, c0:c0 + cw],
                                      in1=bout_rep[:, c0:c0 + cw],
                                      op=AluOp.add)
                    sync.dma_start(
                        out=out[tt * 128:(tt + 1) * 128, c0:c0 + cw],
                        in_=y[tt][:, c0:c0 + cw])
        f_base += ng

    pool_w2.release()
    pool_h.release()
    pool_s4.release()
    pool_y.release()
    persist.release()
    psC.release()
    psD.release()


_cached_nc = None


def _get_nc():
    global _cached_nc
    if _cached_nc is None:
        _cached_nc = _build()
    return _cached_nc


def _host_masks(j):
    """Multiplicative masks: 1 where causal-valid, 0 where invalid.

    Slots 0..6 gate the main (post-AllGather) loop: ones for fully-valid
    super-chunks (s < p), zero otherwise — the diagonal super-chunk s == p is
    computed in the pre-pass from local k/v. Slot 7 holds the causal triangle
    used by that diagonal pre-pass (identical on every core)."""
    m = np.zeros((2, NSC, 2, LC, CH), np.float32)
    l_idx = np.arange(LC)[:, None]
    q_idx = np.arange(CH)[None, :]
    m1 = (l_idx <= q_idx).astype(np.float32)
    m2 = (l_idx + LC <= q_idx).astype(np.float32)
    for c in range(2):
        p = j if c == 0 else 7 - j
        for s in range(NSC - 1):
            if s < p:
                m[c, s, :, :, :] = 1.0
        m[c, 7, 0] = m1
        m[c, 7, 1] = m2
    return m


def _bf16(a):
    import jax.numpy as jnp
    return np.asarray(jnp.asarray(np.asarray(a, np.float32), jnp.bfloat16))


def kernel(x, g1, b1, w_qkv, bias_qkv, w_proj, bias_proj, g2, b2, w_fc,
           bias_fc, w_out, bias_out):
    x = np.asarray(x, np.float32)
    xf = x.reshape(B * S, D)

    # fold LN1 affine into qkv weights; pre-scale q by c^-0.5
    wqkv_m = (np.asarray(w_qkv) * np.asarray(g1)[:, None]).astype(np.float32)
    bqkv_m = (np.asarray(bias_qkv) + np.asarray(b1) @ np.asarray(w_qkv)).astype(
        np.float32)
    sc = 1.0 / np.sqrt(C)
    wqkv_m[:, :D] *= sc
    bqkv_m[:D] *= sc
    wfc_m = (np.asarray(w_fc) * np.asarray(g2)[:, None]).astype(np.float32)
    bfc_m = (np.asarray(bias_fc) + np.asarray(b2) @ np.asarray(w_fc)).astype(
        np.float32)

    common = {
        "wqkv": np.ascontiguousarray(_bf16(wqkv_m)),
        "bqkv": np.ascontiguousarray(bqkv_m),
        "wproj": np.ascontiguousarray(_bf16(w_proj)),
        "bproj": np.ascontiguousarray(np.asarray(bias_proj, np.float32)),
        "wfc": np.ascontiguousarray(_bf16(wfc_m)),
        "bfc": np.ascontiguousarray(bfc_m),
        "wout": np.ascontiguousarray(_bf16(w_out)),
        "bout": np.ascontiguousarray(np.asarray(bias_out, np.float32)),
    }
    in_maps = []
    for j in range(N_CORES):
        a0 = CH * j
        b0 = S + CH * (7 - j)
        xl = np.concatenate([xf[a0:a0 + CH], xf[b0:b0 + CH]], axis=0)
        in_maps.append({
            "x": np.ascontiguousarray(xl),
            "masks": np.ascontiguousarray(_bf16(_host_masks(j))),
            **common,
        })

    nc = _get_nc()
    res = run_bass_kernel_spmd(nc, in_maps, core_ids=list(range(N_CORES)))

    of = np.empty((B * S, D), np.float32)
    for j in range(N_CORES):
        o = res.results[j]["out"]
        a0 = CH * j
        b0 = S + CH * (7 - j)
        of[a0:a0 + CH] = o[:CH]
        of[b0:b0 + CH] = o[CH:]
    return of.reshape(B, S, D)


# revision 86
# speedup vs baseline: 1.0008x; 1.0008x over previous
"""Trainium2 Bass kernel for nn_Block_19069654794616 (dense transformer block).

B=2, S=2048, D=1600, 25 heads x 64, causal attention, 4x MLP (tanh-gelu),
pre-LN with residuals. fp32 in/out, bf16 matmul operands internally.

Distribution (8 NeuronCores, token-parallel):
  Core j owns 512 tokens: chunk A = seq0[256j:256j+256], chunk B =
  seq1[256(7-j):256(8-j)] (mirrored pairing balances the causal triangle).
  - LN1 + QKV per-core on its own tokens (LN gains folded into the QKV
    weights host-side; q pre-scaled by c^-0.5). All weights bf16.
  - k^T / v shards (bf16, v padded with a per-head ones column for the
    softmax denominator) AllGather'd across the 8 cores.
  - Attention: all 25 heads for the 2 query chunks over the full 2048-token
    prefix. Causal/validity masking is a multiplicative 0/1 bf16 mask applied
    on the DVE after exp. AV accumulates in PSUM across all 8 key
    super-chunks (one bank per head pair).
  - proj / LN2 / MLP token-local; outputs concatenated host-side.
"""

import numpy as np

import concourse.bass as bass
import concourse.mybir as mybir
import concourse.tile as tile
from concourse import bacc
from concourse.bass_utils import run_bass_kernel_spmd
from concourse.masks import make_identity

f32 = mybir.dt.float32
bf16 = mybir.dt.bfloat16

N_CORES = 8
B, S, D = 2, 2048, 1600
H, C = 25, 64
D3, D4 = 3 * D, 4 * D
TOK = 512          # tokens per core
CH = 256           # query chunk (2 per core)
LC = 128           # key sub-block
NHP = 13           # head-pair tiles (12 pairs + head 24)
NSC = 8            # key super-chunks (256 tok) per sequence
EPS = 1e-5
VW = 65            # per-head v width (64 + ones column)

# D contraction chunks: 12x128 + 1x64
DCH = [(t * 128, 128) for t in range(12)] + [(1536, 64)]
# output-column tiles of 400 for D-sized outputs (proj / mlp out)
NJ = [(j * 400, 400) for j in range(4)]
# v-proj blocks: 5 heads (320 cols) each
VB = [(b * 5, 5) for b in range(5)]
# mlp f-tile groups (of 50 x 128-col tiles); even pair counts
GRP = [14, 12, 12, 12]

KREG = NHP * 128 * TOK          # bf16 elems in the k^T region of a shard
VREG = 4 * 128 * (H * VW)      # bf16 elems in the v region (ones-padded)
SHARD = KREG + VREG
SHARDP = SHARD                  # slot pitch in kv_all (collective output
                                # must be contiguous per the BIR verifier)
VROW = H * VW                   # 1625


def _build():
    nc = bacc.Bacc(
        "TRN2",
        target_bir_lowering=False,
        debug=False,
        enable_asserts=True,
        num_devices=N_CORES,
    )
    x_in = nc.dram_tensor("x", [TOK, D], f32, kind="ExternalInput").ap()
    wqkv = nc.dram_tensor("wqkv", [D, D3], bf16, kind="ExternalInput").ap()
    bqkv = nc.dram_tensor("bqkv", [D3], f32, kind="ExternalInput").ap()
    wproj = nc.dram_tensor("wproj", [D, D], bf16, kind="ExternalInput").ap()
    bproj = nc.dram_tensor("bproj", [D], f32, kind="ExternalInput").ap()
    wfc = nc.dram_tensor("wfc", [D, D4], bf16, kind="ExternalInput").ap()
    bfc = nc.dram_tensor("bfc", [D4], f32, kind="ExternalInput").ap()
    wout = nc.dram_tensor("wout", [D4, D], bf16, kind="ExternalInput").ap()
    bout = nc.dram_tensor("bout", [D], f32, kind="ExternalInput").ap()
    masks = nc.dram_tensor("masks", [2, NSC, 2, LC, CH], bf16,
                           kind="ExternalInput").ap()
    out = nc.dram_tensor("out", [TOK, D], f32, kind="ExternalOutput").ap()

    shard = nc.dram_tensor("shard", [1, SHARD], bf16, kind="Internal").ap()
    stg_dram = nc.dram_tensor("stg_dram", [NHP, 2, 2, VW, CH], bf16,
                              kind="Internal").ap()
    kv_all = nc.dram_tensor(
        "kv_all", [N_CORES, SHARDP], bf16, kind="Internal", addr_space="Shared"
    ).ap()

    with tile.TileContext(nc, pool_alloc_mode="queue") as tc:
        _emit(tc, nc, x_in, wqkv, bqkv, wproj, bproj, wfc, bfc, wout, bout,
              masks, out, shard, kv_all, stg_dram)
    nc.compile()
    return nc


def _emit(tc, nc, x_in, wqkv, bqkv, wproj, bproj, wfc, bfc, wout, bout,
          masks, out, shard, kv_all, stg_dram):
    sync, vec, act, gp, te = nc.sync, nc.vector, nc.scalar, nc.gpsimd, nc.tensor
    AluOp = mybir.AluOpType
    Act = mybir.ActivationFunctionType

    # ---------------- whole-kernel pools ----------------
    psB = tc.alloc_tile_pool(name="psB", bufs=4, space="PSUM")  # qkv-era
    persist = tc.alloc_tile_pool(name="persist", bufs=1)

    identb = persist.tile([128, 128], bf16, name="identb")
    make_identity(nc, identb)
    eps_t = persist.tile([128, 1], f32, name="eps_t")
    vec.memset(eps_t, EPS)

    # per-partition bias columns for k / q / fc (out-channel on partitions)
    bk_col = persist.tile([128, NHP], f32, name="bk_col")
    bq_col = persist.tile([128, NHP], f32, name="bq_col")
    bfc_col = persist.tile([128, 50], f32, name="bfc_col")
    gp.dma_start(out=bq_col, in_=bass.AP(
        tensor=bqkv.tensor, offset=0, ap=[[1, 128], [128, NHP]]))
    gp.dma_start(out=bk_col, in_=bass.AP(
        tensor=bqkv.tensor, offset=D, ap=[[1, 128], [128, NHP]]))
    gp.dma_start(out=bfc_col, in_=bass.AP(
        tensor=bfc.tensor, offset=0, ap=[[1, 128], [128, 50]]))
    # replicated (per-free) bias rows
    bv_rep = persist.tile([128, D], f32, name="bv_rep")
    bproj_rep = persist.tile([128, D], f32, name="bproj_rep")
    bout_rep = persist.tile([128, D], f32, name="bout_rep")
    act.dma_start(out=bv_rep, in_=bass.AP(
        tensor=bqkv.tensor, offset=2 * D, ap=[[0, 128], [1, D]]))
    act.dma_start(out=bproj_rep, in_=bass.AP(
        tensor=bproj.tensor, offset=0, ap=[[0, 128], [1, D]]))
    act.dma_start(out=bout_rep, in_=bass.AP(
        tensor=bout.tensor, offset=0, ap=[[0, 128], [1, D]]))

    def load_w_cols(pool, src, ncols, col0, cw, name, tag, bufs=2, eng=None):
        """(1600, cw) weight slice -> (128, 13, cw) bf16 tile (2 DMAs)."""
        w = pool.tile([128, NHP, cw], bf16, name=name, tag=tag, bufs=bufs)
        src_t = src.tensor
        e = eng or sync
        e.dma_start(out=w[:, 0:12, :], in_=bass.AP(
            tensor=src_t, offset=col0,
            ap=[[ncols, 128], [128 * ncols, 12], [1, cw]]))
        e.dma_start(out=w[:64, 12:13, :], in_=bass.AP(
            tensor=src_t, offset=1536 * ncols + col0,
            ap=[[ncols, 64], [128 * ncols, 1], [1, cw]]))
        return w

    def ln_transpose(get_src, dst_tiles, pool, label, pspool):
        """get_src(tt) -> (128, D) fp32 SBUF tile; LN + transpose into
        13 (128, TOK) bf16 dst tiles (columns tt*128..)."""
        for tt in range(4):
            xt = get_src(tt)
            stats = pool.tile([128, 4, 6], f32, name=f"{label}st{tt}",
                              tag=f"{label}st")
            xg = xt.rearrange("p (g d) -> p g d", g=4)
            for g in range(4):
                vec.bn_stats(out=stats[:, g, :], in_=xg[:, g, :])
            mv = pool.tile([128, 2], f32, name=f"{label}mv{tt}", tag=f"{label}mv")
            vec.bn_aggr(out=mv, in_=stats)
            rstd = pool.tile([128, 1], f32, name=f"{label}rs{tt}",
                             tag=f"{label}rs")
            act.activation(out=rstd, in_=mv[:, 1:2], func=Act.Sqrt, bias=eps_t)
            vec.reciprocal(out=rstd, in_=rstd)
            xc = pool.tile([128, D], bf16, name=f"{label}xc{tt}",
                           tag=f"{label}xc")
            vec.tensor_scalar(out=xc, in0=xt, scalar1=mv[:, 0:1], scalar2=rstd,
                              op0=AluOp.subtract, op1=AluOp.mult)
            for t, (d0, dp) in enumerate(DCH):
                tp = pspool.tile([128, 128], bf16, name=f"{label}tp",
                                 tag="lntp", bufs=2)
                te.transpose(tp[:dp, :], xc[:, d0:d0 + dp], identb)
                act.activation(out=dst_tiles[t][:dp, tt * 128:(tt + 1) * 128],
                               in_=tp[:dp, :], func=Act.Copy)

    # ======== S1: LN1 -> xcT; k,v; shard; AllGather; q ========
    pool_y = tc.alloc_tile_pool(name="pool_y", bufs=1)
    y = [pool_y.tile([128, D], f32, name=f"y{tt}", tag=f"y{tt}")
         for tt in range(4)]
    pool_qT = tc.alloc_tile_pool(name="pool_qT", bufs=1)
    qT = [pool_qT.tile([128, TOK], bf16, name=f"qT{t}", tag=f"qT{t}")
          for t in range(NHP)]
    pool_s1 = tc.alloc_tile_pool(name="pool_s1", bufs=1)
    kT = [pool_s1.tile([128, TOK], bf16, name=f"kT{t}", tag=f"kT{t}")
          for t in range(NHP)]
    vown = [pool_s1.tile([128, H, VW], bf16, name=f"vown{tt}", tag=f"vown{tt}")
            for tt in range(4)]
    pool_xc = tc.alloc_tile_pool(name="pool_xc", bufs=1)
    xcT = [pool_xc.tile([128, TOK], bf16, name=f"xcT{t}", tag=f"xcT{t}")
           for t in range(NHP)]
    for tt in range(4):
        # ones columns for the softmax denominator (never overwritten)
        gp.memset(vown[tt][:, :, 64:65], 1.0)

    pool_ln = tc.alloc_tile_pool(name="pool_ln", bufs=2)

    def ln1_src(tt):
        xt = pool_ln.tile([128, D], f32, name=f"ln1x{tt}", tag="ln1x")
        for g in range(4):
            sync.dma_start(out=xt[:, g * 400:(g + 1) * 400],
                           in_=x_in[tt * 128:(tt + 1) * 128,
                                    g * 400:(g + 1) * 400])
        return xt

    ln_transpose(ln1_src, xcT, pool_ln, "ln1", pspool=psB)
    pool_ln.release()

    pool_w1 = tc.alloc_tile_pool(name="pool_w1", bufs=1)
    kreg = shard[0, 0:KREG].rearrange("(t p n) -> t p n", t=NHP, p=128)
    vreg = shard[0, KREG:SHARD].rearrange("(t p n) -> t p n", t=4, p=128)

    def qk_proj(col_base, bias_col, dst, to_shard):
        for tp in range(7):
            nt = 2 if tp < 6 else 1
            w = load_w_cols(pool_w1, wqkv, D3, col_base + tp * 256, nt * 128,
                            f"w{col_base}_{tp}", tag="wbig", bufs=3)
            for ti in range(nt):
                t = 2 * tp + ti
                ps = psB.tile([128, TOK], f32, name="qkps", tag="ps1", bufs=2)
                for ci, (d0, dp) in enumerate(DCH):
                    te.matmul(ps, lhsT=w[:dp, ci, ti * 128:(ti + 1) * 128],
                              rhs=xcT[ci][:dp, :],
                              start=(ci == 0), stop=(ci == len(DCH) - 1))
                act.activation(out=dst[t], in_=ps, func=Act.Identity,
                               bias=bias_col[:, t:t + 1])
                if to_shard:
                    act.dma_start(out=kreg[t], in_=dst[t])

    qk_proj(D, bk_col, kT, True)          # k^T -> shard

    # v (natural layout, ones-padded per head) + bias
    vps = {}
    for b0, nb in VB:
        wv = load_w_cols(pool_w1, wqkv, D3, 2 * D + b0 * 64, 320,
                         f"wv{b0}", tag="wv", bufs=2)
        for ci, (d0, dp) in enumerate(DCH):
            for tt in range(4):
                if ci == 0:
                    vps[tt] = psB.tile([128, 320], f32, name=f"vps{tt}",
                                       tag="vps")
                te.matmul(vps[tt], lhsT=xcT[ci][:dp, tt * 128:(tt + 1) * 128],
                          rhs=wv[:dp, ci, :], start=(ci == 0),
                          stop=(ci == len(DCH) - 1))
        for tt in range(4):
            vec.tensor_tensor(
                out=vown[tt][:, b0:b0 + nb, 0:64],
                in0=vps[tt].rearrange("p (h c) -> p h c", h=nb),
                in1=bv_rep[:, b0 * 64:(b0 + nb) * 64].rearrange(
                    "p (h c) -> p h c", h=nb),
                op=AluOp.add)
    for tt in range(4):
        act.dma_start(out=vreg[tt], in_=vown[tt])

    gp.collective_compute(
        "AllGather", mybir.AluOpType.bypass,
        replica_groups=[list(range(N_CORES))],
        ins=[shard], outs=[kv_all[:, 0:SHARD]],
    )

    qk_proj(0, bq_col, qT, False)          # q^T (pre-scaled host-side)

    pool_w1.release()
    pool_xc.release()

    # ======== attention ========
    psB.release()
    psSt = tc.alloc_tile_pool(name="psSt", bufs=2, space="PSUM")   # 4 banks
    psAv = tc.alloc_tile_pool(name="psAv", bufs=2, space="PSUM")   # 2 banks
    psP = tc.alloc_tile_pool(name="psP", bufs=2, space="PSUM")     # 2 banks

    # ---- diagonal pre-pass: own-core k/v (still in SBUF), runs inside the
    # AllGather window; partial AV + denominators staged to DRAM. ----
    pool_pre = tc.alloc_tile_pool(name="pool_pre", bufs=2)
    mdiag = []
    for c in range(2):
        m = pool_pre.tile([128, 2, 2, CH], bf16, name=f"md{c}", tag=f"md{c}",
                          bufs=1)
        for hh in range(2):
            sync.dma_start(out=m[:, hh, :, :], in_=bass.AP(
                tensor=masks.tensor, offset=(c * NSC + 7) * 2 * LC * CH,
                ap=[[CH, 128], [LC * CH, 2], [1, CH]]))
        mdiag.append(m.rearrange("p a b q -> p (a b) q"))
    for hp in range(NHP):
        nh = 1 if hp == 12 else 2
        for c in range(2):
            toff = c * CH
            avd = psAv.tile([VW, 2, CH], f32, name="avd", tag="av")
            st = psSt.tile([128, 4, CH], f32, name="std", tag="st")
            for hh in range(nh):
                p0 = hh * 64
                for lc in range(2):
                    te.matmul(st[:, hh * 2 + lc, :],
                              lhsT=kT[hp][p0:p0 + 64,
                                          toff + lc * LC:toff + (lc + 1) * LC],
                              rhs=qT[hp][p0:p0 + 64, toff:toff + CH],
                              start=(lc == 0), stop=(lc == 1))
            ptm = pool_pre.tile([128, 4, CH], bf16, name="ptmd", tag="ptmd")
            act.activation(out=ptm[:, 0:2 * nh, :], in_=st[:, 0:2 * nh, :],
                           func=Act.Exp)
            vec.tensor_tensor(out=ptm[:, 0:2 * nh, :],
                              in0=ptm[:, 0:2 * nh, :],
                              in1=mdiag[c][:, 0:2 * nh, :], op=AluOp.mult)
            for hh in range(nh):
                h = 2 * hp + hh
                for lc in range(2):
                    te.matmul(avd[:, hh, :],
                              lhsT=vown[2 * c + lc][:, h, :],
                              rhs=ptm[:, hh * 2 + lc, :],
                              start=(hh == 0 and lc == 0),
                              stop=(hh == nh - 1 and lc == 1))
            sb = pool_pre.tile([VW, 2, CH], bf16, name="sb", tag="sb")
            vec.tensor_copy(out=sb[:, 0:nh, :], in_=avd[:, 0:nh, :])
            act.dma_start(out=stg_dram[hp, c], in_=sb)
    pool_pre.release()
    pool_s1.release()

    pool_at = tc.alloc_tile_pool(name="pool_at", bufs=1)
    attn_T = [pool_at.tile([128, TOK], bf16, name=f"attnT{t}", tag=f"attnT{t}")
              for t in range(NHP)]
    pool_msk = tc.alloc_tile_pool(name="pool_msk", bufs=1)
    pool_kv = tc.alloc_tile_pool(name="pool_kv", bufs=4)
    pool_ptm = tc.alloc_tile_pool(name="pool_ptm", bufs=4)
    pool_nrm = tc.alloc_tile_pool(name="pool_nrm", bufs=4)
    pool_pj = tc.alloc_tile_pool(name="pool_pj", bufs=2)

    mkk = []
    for c in range(2):
        row = []
        for s in range(NSC - 1):
            m = pool_msk.tile([128, 2, 2, CH], bf16, name=f"mk{c}_{s}",
                              tag=f"mk{c}_{s}")
            for hh in range(2):
                sync.dma_start(out=m[:, hh, :, :], in_=bass.AP(
                    tensor=masks.tensor, offset=(c * NSC + s) * 2 * LC * CH,
                    ap=[[CH, 128], [LC * CH, 2], [1, CH]]))
            row.append(m.rearrange("p a b q -> p (a b) q"))
        mkk.append(row)

    for c in range(2):
        toff = c * CH
        mk = mkk[c]
        for hp in range(NHP):
            nh = 1 if hp == 12 else 2
            ktg = pool_kv.tile([128, NSC, CH], bf16, name="ktg", tag="ktg")
            # first iteration rides HWDGE so the post-gather ramp is not
            # serialized on Pool SWDGE descriptor generation
            kveng = sync if (c == 0 and hp == 0) else gp
            kveng.dma_start(out=ktg, in_=bass.AP(
                tensor=kv_all.tensor, offset=hp * 128 * TOK + toff,
                ap=[[TOK, 128], [SHARDP, NSC], [1, CH]]))
            vtg = pool_kv.tile([128, NSC, 2, 2 * VW], bf16, name="vtg",
                               tag="vtg")
            for lc in range(2):
                kveng.dma_start(out=vtg[:, :, lc, 0:nh * VW], in_=bass.AP(
                    tensor=kv_all.tensor,
                    offset=KREG + (2 * c + lc) * 128 * VROW + hp * 2 * VW,
                    ap=[[VROW, 128], [SHARDP, NSC], [1, nh * VW]]))
            stgi = pool_kv.tile([VW, 2, CH], bf16, name="stgi", tag="stgi",
                                bufs=3)
            gp.dma_start(out=stgi, in_=stg_dram[hp, c])
            av = psAv.tile([VW, 2, CH], f32, name="av", tag="av")
            # seed the accumulator with the pre-computed diagonal partial
            for hh in range(nh):
                te.matmul(av[:, hh, :], lhsT=identb[0:VW, 0:VW],
                          rhs=stgi[:, hh, :],
                          start=(hh == 0), stop=False)
            for s in range(NSC - 1):
                r = s if c == 0 else 7 - s
                st = psSt.tile([128, 4, CH], f32, name="st", tag="st")
                for hh in range(nh):
                    p0 = hh * 64
                    for lc in range(2):
                        te.matmul(st[:, hh * 2 + lc, :],
                                  lhsT=ktg[p0:p0 + 64, r, lc * LC:(lc + 1) * LC],
                                  rhs=qT[hp][p0:p0 + 64, toff:toff + CH],
                                  start=(lc == 0), stop=(lc == 1))
                ptm = pool_ptm.tile([128, 4, CH], bf16, name="ptm", tag="ptm")
                act.activation(out=ptm[:, 0:2 * nh, :], in_=st[:, 0:2 * nh, :],
                               func=Act.Exp)
                vec.tensor_tensor(out=ptm[:, 0:2 * nh, :],
                                  in0=ptm[:, 0:2 * nh, :],
                                  in1=mk[s][:, 0:2 * nh, :], op=AluOp.mult)
                for hh in range(nh):
                    for lc in range(2):
                        te.matmul(av[:, hh, :],
                                  lhsT=vtg[:, r, lc, hh * VW:(hh + 1) * VW],
                                  rhs=ptm[:, hh * 2 + lc, :],
                                  start=False,
                                  stop=(s == NSC - 2 and hh == nh - 1
                                        and lc == 1))
            for hh in range(nh):
                rcp = pool_nrm.tile([1, CH], f32, name="rcp", tag="rcp")
                vec.reciprocal(out=rcp, in_=av[64:65, hh, :])
                rcpb = pool_nrm.tile([64, CH], f32, name="rcpb", tag="rcpb")
                gp.partition_broadcast(rcpb, rcp)
                vec.tensor_tensor(
                    out=attn_T[hp][hh * 64:(hh + 1) * 64, toff:toff + CH],
                    in0=av[0:64, hh, :], in1=rcpb, op=AluOp.mult)

        # ---- proj + residual for this chunk's 2 token tiles ----
        pps = {}
        for j0, (c0, cw) in enumerate(NJ):
            wp = pool_pj.tile([128, NHP, cw], bf16, name=f"wp{j0}", tag="wp")
            sync.dma_start(out=wp[:, 0:12, :], in_=bass.AP(
                tensor=wproj.tensor, offset=c0,
                ap=[[D, 128], [128 * D, 12], [1, cw]]))
            sync.dma_start(out=wp[:64, 12:13, :], in_=bass.AP(
                tensor=wproj.tensor, offset=1536 * D + c0,
                ap=[[D, 64], [128 * D, 1], [1, cw]]))
            for tt in (2 * c, 2 * c + 1):
                pps[tt] = psP.tile([128, cw], f32, name=f"pps{tt}", tag="pps")
                for ci, (d0, dp) in enumerate(DCH):
                    te.matmul(pps[tt],
                              lhsT=attn_T[ci][:dp, tt * 128:(tt + 1) * 128],
                              rhs=wp[:dp, ci, :], start=(ci == 0),
                              stop=(ci == len(DCH) - 1))
                xr = pool_pj.tile([128, cw], f32, name=f"xr{tt}", tag="xr",
                                  bufs=3)
                sync.dma_start(out=xr,
                               in_=x_in[tt * 128:(tt + 1) * 128, c0:c0 + cw])
                vec.tensor_tensor(out=y[tt][:, c0:c0 + cw], in0=pps[tt],
                                  in1=xr, op=AluOp.add)
                vec.tensor_tensor(out=y[tt][:, c0:c0 + cw],
                                  in0=y[tt][:, c0:c0 + cw],
                                  in1=bproj_rep[:, c0:c0 + cw], op=AluOp.add)

    pool_pj.release()
    pool_nrm.release()
    pool_ptm.release()
    pool_kv.release()
    pool_msk.release()
    pool_at.release()
    pool_qT.release()
    psP.release()
    psAv.release()
    psSt.release()

    # ======== LN2 -> ycT; MLP; out ========
    psT = tc.alloc_tile_pool(name="psT", bufs=2, space="PSUM")
    pool_s4 = tc.alloc_tile_pool(name="pool_s4", bufs=1)
    ycT = [pool_s4.tile([128, TOK], bf16, name=f"ycT{t}", tag=f"ycT{t}")
           for t in range(NHP)]
    pool_ln2 = tc.alloc_tile_pool(name="pool_ln2", bufs=2)
    ln_transpose(lambda tt: y[tt], ycT, pool_ln2, "ln2", pspool=psT)
    pool_ln2.release()
    psT.release()
    psD = tc.alloc_tile_pool(name="psD", bufs=3, space="PSUM")
    psC = tc.alloc_tile_pool(name="psC", bufs=4, space="PSUM")

    pool_h = tc.alloc_tile_pool(name="pool_h", bufs=2)
    pool_w2 = tc.alloc_tile_pool(name="pool_w2", bufs=1)

    ops = {}
    f_base = 0
    for ng in GRP:
        hT = [pool_h.tile([128, TOK], bf16, name=f"hT{f_base}_{fi}",
                          tag=f"hT{fi}") for fi in range(ng)]
        for fp in range(ng // 2):
            wf = load_w_cols(pool_w2, wfc, D4, (f_base + 2 * fp) * 128, 256,
                             f"wf{f_base}_{fp}", tag="wf", bufs=4)
            for ti in range(2):
                fi = 2 * fp + ti
                f = f_base + fi
                ps = psD.tile([128, TOK], f32, name="hps", tag="hps")
                for ci, (d0, dp) in enumerate(DCH):
                    te.matmul(ps, lhsT=wf[:dp, ci, ti * 128:(ti + 1) * 128],
                              rhs=ycT[ci][:dp, :],
                              start=(ci == 0), stop=(ci == len(DCH) - 1))
                act.activation(out=hT[fi], in_=ps, func=Act.Gelu_apprx_tanh,
                               bias=bfc_col[:, f:f + 1], scale=1.0)
        for j0, (c0, cw) in enumerate(NJ):
            wo = pool_w2.tile([128, ng, cw], bf16, name=f"wo{f_base}_{j0}",
                              tag="wo", bufs=2)
            sync.dma_start(out=wo, in_=bass.AP(
                tensor=wout.tensor, offset=f_base * 128 * D + c0,
                ap=[[D, 128], [128 * D, ng], [1, cw]]))
            for tt in range(4):
                ops[tt] = psC.tile([128, cw], f32, name=f"ops{tt}", tag="ops")
                for fi in range(ng):
                    te.matmul(ops[tt], lhsT=hT[fi][:, tt * 128:(tt + 1) * 128],
                              rhs=wo[:, fi, :], start=(fi == 0),
                              stop=(fi == ng - 1))
            last = (f_base + ng == 50)
            for tt in range(4):
                vec.tensor_tensor(out=y[tt][:, c0:c0 + cw],
                                  in0=y[tt][:, c0:c0 + cw], in1=ops[tt],
                                  op=AluOp.add)
                if last:
                    vec.tensor_tensor(out=y[tt][:, c0:c0 + cw],
                                      in0=y[tt][:, c0:c0 + cw],
                                      in1=bout_rep[:, c0:c0 + cw],
                                      op=AluOp.add)
                    sync.dma_start(
                        out=out[tt * 128:(tt + 1) * 128, c0:c0 + cw],
                        in_=y[tt][:, c0:c0 + cw])
        f_base += ng

    pool_w2.release()
    pool_h.release()
    pool_s4.release()
    pool_y.release()
    persist.release()
    psC.release()
    psD.release()


_cached_nc = None


def _get_nc():
    global _cached_nc
    if _cached_nc is None:
        _cached_nc = _build()
    return _cached_nc


def _host_masks(j):
    """Multiplicative masks: 1 where causal-valid, 0 where invalid.

    Slots 0..6 gate the main (post-AllGather) loop: ones for fully-valid
    super-chunks (s < p), zero otherwise — the diagonal super-chunk s == p is
    computed in the pre-pass from local k/v. Slot 7 holds the causal triangle
    used by that diagonal pre-pass (identical on every core)."""
    m = np.zeros((2, NSC, 2, LC, CH), np.float32)
    l_idx = np.arange(LC)[:, None]
    q_idx = np.arange(CH)[None, :]
    m1 = (l_idx <= q_idx).astype(np.float32)
    m2 = (l_idx + LC <= q_idx).astype(np.float32)
    for c in range(2):
        p = j if c == 0 else 7 - j
        for s in range(NSC - 1):
            if s < p:
                m[c, s, :, :, :] = 1.0
        m[c, 7, 0] = m1
        m[c, 7, 1] = m2
    return m


def _bf16(a):
    import jax.numpy as jnp
    return np.asarray(jnp.asarray(np.asarray(a, np.float32), jnp.bfloat16))


def kernel(x, g1, b1, w_qkv, bias_qkv, w_proj, bias_proj, g2, b2, w_fc,
           bias_fc, w_out, bias_out):
    x = np.asarray(x, np.float32)
    xf = x.reshape(B * S, D)

    # fold LN1 affine into qkv weights; pre-scale q by c^-0.5
    wqkv_m = (np.asarray(w_qkv) * np.asarray(g1)[:, None]).astype(np.float32)
    bqkv_m = (np.asarray(bias_qkv) + np.asarray(b1) @ np.asarray(w_qkv)).astype(
        np.float32)
    sc = 1.0 / np.sqrt(C)
    wqkv_m[:, :D] *= sc
    bqkv_m[:D] *= sc
    wfc_m = (np.asarray(w_fc) * np.asarray(g2)[:, None]).astype(np.float32)
    bfc_m = (np.asarray(bias_fc) + np.asarray(b2) @ np.asarray(w_fc)).astype(
        np.float32)

    common = {
        "wqkv": np.ascontiguousarray(_bf16(wqkv_m)),
        "bqkv": np.ascontiguousarray(bqkv_m),
        "wproj": np.ascontiguousarray(_bf16(w_proj)),
        "bproj": np.ascontiguousarray(np.asarray(bias_proj, np.float32)),
        "wfc": np.ascontiguousarray(_bf16(wfc_m)),
        "bfc": np.ascontiguousarray(bfc_m),
        "wout": np.ascontiguousarray(_bf16(w_out)),
        "bout": np.ascontiguousarray(np.asarray(bias_out, np.float32)),
    }
    in_maps = []
    for j in range(N_CORES):
        a0 = CH * j
        b0 = S + CH * (7 - j)
        xl = np.concatenate([xf[a0:a0 + CH], xf[b0:b0 + CH]], axis=0)
        in_maps.append({
            "x": np.ascontiguousarray(xl),
            "masks": np.ascontiguousarray(_bf16(_host_masks(j))),
            **common,
        })

    nc = _get_nc()
    res = run_bass_kernel_spmd(nc, in_maps, core_ids=list(range(N_CORES)))

    of = np.empty((B * S, D), np.float32)
    for j in range(N_CORES):
        o = res.results[j]["out"]
        a0 = CH * j
        b0 = S + CH * (7 - j)
        of[a0:a0 + CH] = o[:CH]
        of[b0:b0 + CH] = o[CH:]
    return of.reshape(B, S, D)


# revision 87
# speedup vs baseline: 1.0011x; 1.0003x over previous
"""Trainium2 Bass kernel for nn_Block_19069654794616 (dense transformer block).

B=2, S=2048, D=1600, 25 heads x 64, causal attention, 4x MLP (tanh-gelu),
pre-LN with residuals. fp32 in/out, bf16 matmul operands internally.

Distribution (8 NeuronCores, token-parallel):
  Core j owns 512 tokens: chunk A = seq0[256j:256j+256], chunk B =
  seq1[256(7-j):256(8-j)] (mirrored pairing balances the causal triangle).
  - LN1 + QKV per-core on its own tokens (LN gains folded into the QKV
    weights host-side; q pre-scaled by c^-0.5). All weights bf16.
  - k^T / v shards (bf16, v padded with a per-head ones column for the
    softmax denominator) AllGather'd across the 8 cores.
  - Attention: all 25 heads for the 2 query chunks over the full 2048-token
    prefix. Causal/validity masking is a multiplicative 0/1 bf16 mask applied
    on the DVE after exp. AV accumulates in PSUM across all 8 key
    super-chunks (one bank per head pair).
  - proj / LN2 / MLP token-local; outputs concatenated host-side.
"""

import numpy as np

import concourse.bass as bass
import concourse.mybir as mybir
import concourse.tile as tile
from concourse import bacc
from concourse.bass_utils import run_bass_kernel_spmd
from concourse.masks import make_identity

f32 = mybir.dt.float32
bf16 = mybir.dt.bfloat16

N_CORES = 8
B, S, D = 2, 2048, 1600
H, C = 25, 64
D3, D4 = 3 * D, 4 * D
TOK = 512          # tokens per core
CH = 256           # query chunk (2 per core)
LC = 128           # key sub-block
NHP = 13           # head-pair tiles (12 pairs + head 24)
NSC = 8            # key super-chunks (256 tok) per sequence
EPS = 1e-5
VW = 65            # per-head v width (64 + ones column)

# D contraction chunks: 12x128 + 1x64
DCH = [(t * 128, 128) for t in range(12)] + [(1536, 64)]
# output-column tiles of 400 for D-sized outputs (proj / mlp out)
NJ = [(j * 400, 400) for j in range(4)]
# v-proj blocks: 5 heads (320 cols) each
VB = [(b * 5, 5) for b in range(5)]
# mlp f-tile groups (of 50 x 128-col tiles); even pair counts
GRP = [14, 12, 12, 12]

KREG = NHP * 128 * TOK          # bf16 elems in the k^T region of a shard
VREG = 4 * 128 * (H * VW)      # bf16 elems in the v region (ones-padded)
SHARD = KREG + VREG
SHARDP = SHARD                  # slot pitch in kv_all (collective output
                                # must be contiguous per the BIR verifier)
VROW = H * VW                   # 1625


def _build():
    nc = bacc.Bacc(
        "TRN2",
        target_bir_lowering=False,
        debug=False,
        enable_asserts=True,
        num_devices=N_CORES,
    )
    x_in = nc.dram_tensor("x", [TOK, D], f32, kind="ExternalInput").ap()
    wqkv = nc.dram_tensor("wqkv", [D, D3], bf16, kind="ExternalInput").ap()
    bqkv = nc.dram_tensor("bqkv", [D3], f32, kind="ExternalInput").ap()
    wproj = nc.dram_tensor("wproj", [D, D], bf16, kind="ExternalInput").ap()
    bproj = nc.dram_tensor("bproj", [D], f32, kind="ExternalInput").ap()
    wfc = nc.dram_tensor("wfc", [D, D4], bf16, kind="ExternalInput").ap()
    bfc = nc.dram_tensor("bfc", [D4], f32, kind="ExternalInput").ap()
    wout = nc.dram_tensor("wout", [D4, D], bf16, kind="ExternalInput").ap()
    bout = nc.dram_tensor("bout", [D], f32, kind="ExternalInput").ap()
    masks = nc.dram_tensor("masks", [2, NSC, 2, LC, CH], bf16,
                           kind="ExternalInput").ap()
    out = nc.dram_tensor("out", [TOK, D], f32, kind="ExternalOutput").ap()

    shard = nc.dram_tensor("shard", [1, SHARD], bf16, kind="Internal").ap()
    stg_dram = nc.dram_tensor("stg_dram", [NHP, 2, 2, VW, CH], bf16,
                              kind="Internal").ap()
    kv_all = nc.dram_tensor(
        "kv_all", [N_CORES, SHARDP], bf16, kind="Internal", addr_space="Shared"
    ).ap()

    with tile.TileContext(nc, pool_alloc_mode="queue") as tc:
        _emit(tc, nc, x_in, wqkv, bqkv, wproj, bproj, wfc, bfc, wout, bout,
              masks, out, shard, kv_all, stg_dram)
    nc.compile()
    return nc


def _emit(tc, nc, x_in, wqkv, bqkv, wproj, bproj, wfc, bfc, wout, bout,
          masks, out, shard, kv_all, stg_dram):
    sync, vec, act, gp, te = nc.sync, nc.vector, nc.scalar, nc.gpsimd, nc.tensor
    AluOp = mybir.AluOpType
    Act = mybir.ActivationFunctionType

    # ---------------- whole-kernel pools ----------------
    psB = tc.alloc_tile_pool(name="psB", bufs=4, space="PSUM")  # qkv-era
    persist = tc.alloc_tile_pool(name="persist", bufs=1)

    identb = persist.tile([128, 128], bf16, name="identb")
    make_identity(nc, identb)
    eps_t = persist.tile([128, 1], f32, name="eps_t")
    vec.memset(eps_t, EPS)

    # per-partition bias columns for k / q / fc (out-channel on partitions)
    bk_col = persist.tile([128, NHP], f32, name="bk_col")
    bq_col = persist.tile([128, NHP], f32, name="bq_col")
    bfc_col = persist.tile([128, 50], f32, name="bfc_col")
    gp.dma_start(out=bq_col, in_=bass.AP(
        tensor=bqkv.tensor, offset=0, ap=[[1, 128], [128, NHP]]))
    gp.dma_start(out=bk_col, in_=bass.AP(
        tensor=bqkv.tensor, offset=D, ap=[[1, 128], [128, NHP]]))
    gp.dma_start(out=bfc_col, in_=bass.AP(
        tensor=bfc.tensor, offset=0, ap=[[1, 128], [128, 50]]))
    # replicated (per-free) bias rows
    bv_rep = persist.tile([128, D], f32, name="bv_rep")
    bproj_rep = persist.tile([128, D], f32, name="bproj_rep")
    bout_rep = persist.tile([128, D], f32, name="bout_rep")
    act.dma_start(out=bv_rep, in_=bass.AP(
        tensor=bqkv.tensor, offset=2 * D, ap=[[0, 128], [1, D]]))
    act.dma_start(out=bproj_rep, in_=bass.AP(
        tensor=bproj.tensor, offset=0, ap=[[0, 128], [1, D]]))
    act.dma_start(out=bout_rep, in_=bass.AP(
        tensor=bout.tensor, offset=0, ap=[[0, 128], [1, D]]))

    def load_w_cols(pool, src, ncols, col0, cw, name, tag, bufs=2, eng=None):
        """(1600, cw) weight slice -> (128, 13, cw) bf16 tile (2 DMAs)."""
        w = pool.tile([128, NHP, cw], bf16, name=name, tag=tag, bufs=bufs)
        src_t = src.tensor
        e = eng or sync
        e.dma_start(out=w[:, 0:12, :], in_=bass.AP(
            tensor=src_t, offset=col0,
            ap=[[ncols, 128], [128 * ncols, 12], [1, cw]]))
        e.dma_start(out=w[:64, 12:13, :], in_=bass.AP(
            tensor=src_t, offset=1536 * ncols + col0,
            ap=[[ncols, 64], [128 * ncols, 1], [1, cw]]))
        return w

    def ln_transpose(get_src, dst_tiles, pool, label, pspool):
        """get_src(tt) -> (128, D) fp32 SBUF tile; LN + transpose into
        13 (128, TOK) bf16 dst tiles (columns tt*128..)."""
        for tt in range(4):
            xt = get_src(tt)
            stats = pool.tile([128, 4, 6], f32, name=f"{label}st{tt}",
                              tag=f"{label}st")
            xg = xt.rearrange("p (g d) -> p g d", g=4)
            for g in range(4):
                vec.bn_stats(out=stats[:, g, :], in_=xg[:, g, :])
            mv = pool.tile([128, 2], f32, name=f"{label}mv{tt}", tag=f"{label}mv")
            vec.bn_aggr(out=mv, in_=stats)
            rstd = pool.tile([128, 1], f32, name=f"{label}rs{tt}",
                             tag=f"{label}rs")
            act.activation(out=rstd, in_=mv[:, 1:2], func=Act.Sqrt, bias=eps_t)
            vec.reciprocal(out=rstd, in_=rstd)
            xc = pool.tile([128, D], bf16, name=f"{label}xc{tt}",
                           tag=f"{label}xc")
            vec.tensor_scalar(out=xc, in0=xt, scalar1=mv[:, 0:1], scalar2=rstd,
                              op0=AluOp.subtract, op1=AluOp.mult)
            for t, (d0, dp) in enumerate(DCH):
                tp = pspool.tile([128, 128], bf16, name=f"{label}tp",
                                 tag="lntp", bufs=2)
                te.transpose(tp[:dp, :], xc[:, d0:d0 + dp], identb)
                act.activation(out=dst_tiles[t][:dp, tt * 128:(tt + 1) * 128],
                               in_=tp[:dp, :], func=Act.Copy)

    # ======== S1: LN1 -> xcT; k,v; shard; AllGather; q ========
    pool_y = tc.alloc_tile_pool(name="pool_y", bufs=1)
    y = [pool_y.tile([128, D], f32, name=f"y{tt}", tag=f"y{tt}")
         for tt in range(4)]
    pool_qT = tc.alloc_tile_pool(name="pool_qT", bufs=1)
    qT = [pool_qT.tile([128, TOK], bf16, name=f"qT{t}", tag=f"qT{t}")
          for t in range(NHP)]
    pool_s1 = tc.alloc_tile_pool(name="pool_s1", bufs=1)
    kT = [pool_s1.tile([128, TOK], bf16, name=f"kT{t}", tag=f"kT{t}")
          for t in range(NHP)]
    vown = [pool_s1.tile([128, H, VW], bf16, name=f"vown{tt}", tag=f"vown{tt}")
            for tt in range(4)]
    pool_xc = tc.alloc_tile_pool(name="pool_xc", bufs=1)
    xcT = [pool_xc.tile([128, TOK], bf16, name=f"xcT{t}", tag=f"xcT{t}")
           for t in range(NHP)]
    for tt in range(4):
        # ones columns for the softmax denominator (never overwritten)
        gp.memset(vown[tt][:, :, 64:65], 1.0)

    pool_ln = tc.alloc_tile_pool(name="pool_ln", bufs=2)

    def ln1_src(tt):
        xt = pool_ln.tile([128, D], f32, name=f"ln1x{tt}", tag="ln1x")
        for g in range(4):
            sync.dma_start(out=xt[:, g * 400:(g + 1) * 400],
                           in_=x_in[tt * 128:(tt + 1) * 128,
                                    g * 400:(g + 1) * 400])
        return xt

    ln_transpose(ln1_src, xcT, pool_ln, "ln1", pspool=psB)
    pool_ln.release()

    pool_w1 = tc.alloc_tile_pool(name="pool_w1", bufs=1)
    kreg = shard[0, 0:KREG].rearrange("(t p n) -> t p n", t=NHP, p=128)
    vreg = shard[0, KREG:SHARD].rearrange("(t p n) -> t p n", t=4, p=128)

    def qk_proj(col_base, bias_col, dst, to_shard):
        for tp in range(7):
            nt = 2 if tp < 6 else 1
            w = load_w_cols(pool_w1, wqkv, D3, col_base + tp * 256, nt * 128,
                            f"w{col_base}_{tp}", tag="wbig", bufs=3)
            for ti in range(nt):
                t = 2 * tp + ti
                ps = psB.tile([128, TOK], f32, name="qkps", tag="ps1", bufs=2)
                for ci, (d0, dp) in enumerate(DCH):
                    te.matmul(ps, lhsT=w[:dp, ci, ti * 128:(ti + 1) * 128],
                              rhs=xcT[ci][:dp, :],
                              start=(ci == 0), stop=(ci == len(DCH) - 1))
                act.activation(out=dst[t], in_=ps, func=Act.Identity,
                               bias=bias_col[:, t:t + 1])
                if to_shard:
                    act.dma_start(out=kreg[t], in_=dst[t])

    qk_proj(D, bk_col, kT, True)          # k^T -> shard

    # v (natural layout, ones-padded per head) + bias
    vps = {}
    for b0, nb in VB:
        wv = load_w_cols(pool_w1, wqkv, D3, 2 * D + b0 * 64, 320,
                         f"wv{b0}", tag="wv", bufs=2)
        for ci, (d0, dp) in enumerate(DCH):
            for tt in range(4):
                if ci == 0:
                    vps[tt] = psB.tile([128, 320], f32, name=f"vps{tt}",
                                       tag="vps")
                te.matmul(vps[tt], lhsT=xcT[ci][:dp, tt * 128:(tt + 1) * 128],
                          rhs=wv[:dp, ci, :], start=(ci == 0),
                          stop=(ci == len(DCH) - 1))
        for tt in range(4):
            vec.tensor_tensor(
                out=vown[tt][:, b0:b0 + nb, 0:64],
                in0=vps[tt].rearrange("p (h c) -> p h c", h=nb),
                in1=bv_rep[:, b0 * 64:(b0 + nb) * 64].rearrange(
                    "p (h c) -> p h c", h=nb),
                op=AluOp.add)
    for tt in range(4):
        act.dma_start(out=vreg[tt], in_=vown[tt])

    gp.collective_compute(
        "AllGather", mybir.AluOpType.bypass,
        replica_groups=[list(range(N_CORES))],
        ins=[shard], outs=[kv_all[:, 0:SHARD]],
    )

    qk_proj(0, bq_col, qT, False)          # q^T (pre-scaled host-side)

    pool_w1.release()
    pool_xc.release()

    # ======== attention ========
    psB.release()
    psSt = tc.alloc_tile_pool(name="psSt", bufs=2, space="PSUM")   # 4 banks
    psAv = tc.alloc_tile_pool(name="psAv", bufs=2, space="PSUM")   # 2 banks
    psP = tc.alloc_tile_pool(name="psP", bufs=2, space="PSUM")     # 2 banks

    # ---- diagonal pre-pass: own-core k/v (still in SBUF), runs inside the
    # AllGather window; partial AV + denominators staged to DRAM. ----
    pool_pre = tc.alloc_tile_pool(name="pool_pre", bufs=2)
    mdiag = []
    for c in range(2):
        m = pool_pre.tile([128, 2, 2, CH], bf16, name=f"md{c}", tag=f"md{c}",
                          bufs=1)
        for hh in range(2):
            sync.dma_start(out=m[:, hh, :, :], in_=bass.AP(
                tensor=masks.tensor, offset=(c * NSC + 7) * 2 * LC * CH,
                ap=[[CH, 128], [LC * CH, 2], [1, CH]]))
        mdiag.append(m.rearrange("p a b q -> p (a b) q"))
    for hp in range(NHP):
        nh = 1 if hp == 12 else 2
        for c in range(2):
            toff = c * CH
            avd = psAv.tile([VW, 2, CH], f32, name="avd", tag="av")
            st = psSt.tile([128, 4, CH], f32, name="std", tag="st")
            for hh in range(nh):
                p0 = hh * 64
                for lc in range(2):
                    te.matmul(st[:, hh * 2 + lc, :],
                              lhsT=kT[hp][p0:p0 + 64,
                                          toff + lc * LC:toff + (lc + 1) * LC],
                              rhs=qT[hp][p0:p0 + 64, toff:toff + CH],
                              start=(lc == 0), stop=(lc == 1))
            ptm = pool_pre.tile([128, 4, CH], bf16, name="ptmd", tag="ptmd")
            act.activation(out=ptm[:, 0:2 * nh, :], in_=st[:, 0:2 * nh, :],
                           func=Act.Exp)
            vec.tensor_tensor(out=ptm[:, 0:2 * nh, :],
                              in0=ptm[:, 0:2 * nh, :],
                              in1=mdiag[c][:, 0:2 * nh, :], op=AluOp.mult)
            for hh in range(nh):
                h = 2 * hp + hh
                for lc in range(2):
                    te.matmul(avd[:, hh, :],
                              lhsT=vown[2 * c + lc][:, h, :],
                              rhs=ptm[:, hh * 2 + lc, :],
                              start=(hh == 0 and lc == 0),
                              stop=(hh == nh - 1 and lc == 1))
            sb = pool_pre.tile([VW, 2, CH], bf16, name="sb", tag="sb")
            vec.tensor_copy(out=sb[:, 0:nh, :], in_=avd[:, 0:nh, :])
            act.dma_start(out=stg_dram[hp, c], in_=sb)
    pool_pre.release()
    pool_s1.release()

    pool_at = tc.alloc_tile_pool(name="pool_at", bufs=1)
    attn_T = [pool_at.tile([128, TOK], bf16, name=f"attnT{t}", tag=f"attnT{t}")
              for t in range(NHP)]
    pool_msk = tc.alloc_tile_pool(name="pool_msk", bufs=1)
    pool_kv = tc.alloc_tile_pool(name="pool_kv", bufs=4)
    pool_ptm = tc.alloc_tile_pool(name="pool_ptm", bufs=4)
    pool_nrm = tc.alloc_tile_pool(name="pool_nrm", bufs=4)
    pool_pj = tc.alloc_tile_pool(name="pool_pj", bufs=2)

    mkk = []
    for c in range(2):
        row = []
        for s in range(NSC - 1):
            m = pool_msk.tile([128, 2, 2, CH], bf16, name=f"mk{c}_{s}",
                              tag=f"mk{c}_{s}")
            for hh in range(2):
                sync.dma_start(out=m[:, hh, :, :], in_=bass.AP(
                    tensor=masks.tensor, offset=(c * NSC + s) * 2 * LC * CH,
                    ap=[[CH, 128], [LC * CH, 2], [1, CH]]))
            row.append(m.rearrange("p a b q -> p (a b) q"))
        mkk.append(row)

    for c in range(2):
        toff = c * CH
        mk = mkk[c]
        for hp in range(NHP):
            nh = 1 if hp == 12 else 2
            ktg = pool_kv.tile([128, NSC, CH], bf16, name="ktg", tag="ktg")
            # first iteration rides HWDGE so the post-gather ramp is not
            # serialized on Pool SWDGE descriptor generation
            kveng = sync if hp == 0 else gp
            kveng.dma_start(out=ktg, in_=bass.AP(
                tensor=kv_all.tensor, offset=hp * 128 * TOK + toff,
                ap=[[TOK, 128], [SHARDP, NSC], [1, CH]]))
            vtg = pool_kv.tile([128, NSC, 2, 2 * VW], bf16, name="vtg",
                               tag="vtg")
            for lc in range(2):
                kveng.dma_start(out=vtg[:, :, lc, 0:nh * VW], in_=bass.AP(
                    tensor=kv_all.tensor,
                    offset=KREG + (2 * c + lc) * 128 * VROW + hp * 2 * VW,
                    ap=[[VROW, 128], [SHARDP, NSC], [1, nh * VW]]))
            stgi = pool_kv.tile([VW, 2, CH], bf16, name="stgi", tag="stgi",
                                bufs=3)
            gp.dma_start(out=stgi, in_=stg_dram[hp, c])
            av = psAv.tile([VW, 2, CH], f32, name="av", tag="av")
            # seed the accumulator with the pre-computed diagonal partial
            for hh in range(nh):
                te.matmul(av[:, hh, :], lhsT=identb[0:VW, 0:VW],
                          rhs=stgi[:, hh, :],
                          start=(hh == 0), stop=False)
            for s in range(NSC - 1):
                r = s if c == 0 else 7 - s
                st = psSt.tile([128, 4, CH], f32, name="st", tag="st")
                for hh in range(nh):
                    p0 = hh * 64
                    for lc in range(2):
                        te.matmul(st[:, hh * 2 + lc, :],
                                  lhsT=ktg[p0:p0 + 64, r, lc * LC:(lc + 1) * LC],
                                  rhs=qT[hp][p0:p0 + 64, toff:toff + CH],
                                  start=(lc == 0), stop=(lc == 1))
                ptm = pool_ptm.tile([128, 4, CH], bf16, name="ptm", tag="ptm")
                act.activation(out=ptm[:, 0:2 * nh, :], in_=st[:, 0:2 * nh, :],
                               func=Act.Exp)
                vec.tensor_tensor(out=ptm[:, 0:2 * nh, :],
                                  in0=ptm[:, 0:2 * nh, :],
                                  in1=mk[s][:, 0:2 * nh, :], op=AluOp.mult)
                for hh in range(nh):
                    for lc in range(2):
                        te.matmul(av[:, hh, :],
                                  lhsT=vtg[:, r, lc, hh * VW:(hh + 1) * VW],
                                  rhs=ptm[:, hh * 2 + lc, :],
                                  start=False,
                                  stop=(s == NSC - 2 and hh == nh - 1
                                        and lc == 1))
            for hh in range(nh):
                rcp = pool_nrm.tile([1, CH], f32, name="rcp", tag="rcp")
                vec.reciprocal(out=rcp, in_=av[64:65, hh, :])
                rcpb = pool_nrm.tile([64, CH], f32, name="rcpb", tag="rcpb")
                gp.partition_broadcast(rcpb, rcp)
                vec.tensor_tensor(
                    out=attn_T[hp][hh * 64:(hh + 1) * 64, toff:toff + CH],
                    in0=av[0:64, hh, :], in1=rcpb, op=AluOp.mult)

        # ---- proj + residual for this chunk's 2 token tiles ----
        pps = {}
        for j0, (c0, cw) in enumerate(NJ):
            wp = pool_pj.tile([128, NHP, cw], bf16, name=f"wp{j0}", tag="wp")
            sync.dma_start(out=wp[:, 0:12, :], in_=bass.AP(
                tensor=wproj.tensor, offset=c0,
                ap=[[D, 128], [128 * D, 12], [1, cw]]))
            sync.dma_start(out=wp[:64, 12:13, :], in_=bass.AP(
                tensor=wproj.tensor, offset=1536 * D + c0,
                ap=[[D, 64], [128 * D, 1], [1, cw]]))
            for tt in (2 * c, 2 * c + 1):
                pps[tt] = psP.tile([128, cw], f32, name=f"pps{tt}", tag="pps")
                for ci, (d0, dp) in enumerate(DCH):
                    te.matmul(pps[tt],
                              lhsT=attn_T[ci][:dp, tt * 128:(tt + 1) * 128],
                              rhs=wp[:dp, ci, :], start=(ci == 0),
                              stop=(ci == len(DCH) - 1))
                xr = pool_pj.tile([128, cw], f32, name=f"xr{tt}", tag="xr",
                                  bufs=3)
                sync.dma_start(out=xr,
                               in_=x_in[tt * 128:(tt + 1) * 128, c0:c0 + cw])
                vec.tensor_tensor(out=y[tt][:, c0:c0 + cw], in0=pps[tt],
                                  in1=xr, op=AluOp.add)
                vec.tensor_tensor(out=y[tt][:, c0:c0 + cw],
                                  in0=y[tt][:, c0:c0 + cw],
                                  in1=bproj_rep[:, c0:c0 + cw], op=AluOp.add)

    pool_pj.release()
    pool_nrm.release()
    pool_ptm.release()
    pool_kv.release()
    pool_msk.release()
    pool_at.release()
    pool_qT.release()
    psP.release()
    psAv.release()
    psSt.release()

    # ======== LN2 -> ycT; MLP; out ========
    psT = tc.alloc_tile_pool(name="psT", bufs=2, space="PSUM")
    pool_s4 = tc.alloc_tile_pool(name="pool_s4", bufs=1)
    ycT = [pool_s4.tile([128, TOK], bf16, name=f"ycT{t}", tag=f"ycT{t}")
           for t in range(NHP)]
    pool_ln2 = tc.alloc_tile_pool(name="pool_ln2", bufs=2)
    ln_transpose(lambda tt: y[tt], ycT, pool_ln2, "ln2", pspool=psT)
    pool_ln2.release()
    psT.release()
    psD = tc.alloc_tile_pool(name="psD", bufs=3, space="PSUM")
    psC = tc.alloc_tile_pool(name="psC", bufs=4, space="PSUM")

    pool_h = tc.alloc_tile_pool(name="pool_h", bufs=2)
    pool_w2 = tc.alloc_tile_pool(name="pool_w2", bufs=1)

    ops = {}
    f_base = 0
    for ng in GRP:
        hT = [pool_h.tile([128, TOK], bf16, name=f"hT{f_base}_{fi}",
                          tag=f"hT{fi}") for fi in range(ng)]
        for fp in range(ng // 2):
            wf = load_w_cols(pool_w2, wfc, D4, (f_base + 2 * fp) * 128, 256,
                             f"wf{f_base}_{fp}", tag="wf", bufs=4)
            for ti in range(2):
                fi = 2 * fp + ti
                f = f_base + fi
                ps = psD.tile([128, TOK], f32, name="hps", tag="hps")
                for ci, (d0, dp) in enumerate(DCH):
                    te.matmul(ps, lhsT=wf[:dp, ci, ti * 128:(ti + 1) * 128],
                              rhs=ycT[ci][:dp, :],
                              start=(ci == 0), stop=(ci == len(DCH) - 1))
                act.activation(out=hT[fi], in_=ps, func=Act.Gelu_apprx_tanh,
                               bias=bfc_col[:, f:f + 1], scale=1.0)
        for j0, (c0, cw) in enumerate(NJ):
            wo = pool_w2.tile([128, ng, cw], bf16, name=f"wo{f_base}_{j0}",
                              tag="wo", bufs=2)
            sync.dma_start(out=wo, in_=bass.AP(
                tensor=wout.tensor, offset=f_base * 128 * D + c0,
                ap=[[D, 128], [128 * D, ng], [1, cw]]))
            for tt in range(4):
                ops[tt] = psC.tile([128, cw], f32, name=f"ops{tt}", tag="ops")
                for fi in range(ng):
                    te.matmul(ops[tt], lhsT=hT[fi][:, tt * 128:(tt + 1) * 128],
                              rhs=wo[:, fi, :], start=(fi == 0),
                              stop=(fi == ng - 1))
            last = (f_base + ng == 50)
            for tt in range(4):
                vec.tensor_tensor(out=y[tt][:, c0:c0 + cw],
                                  in0=y[tt][:, c0:c0 + cw], in1=ops[tt],
                                  op=AluOp.add)
                if last:
                    vec.tensor_tensor(out=y[tt][:, c0:c0 + cw],
                                      in0=y[tt][:, c0:c0 + cw],
                                      in1=bout_rep[:, c0:c0 + cw],
                                      op=AluOp.add)
                    sync.dma_start(
                        out=out[tt * 128:(tt + 1) * 128, c0:c0 + cw],
                        in_=y[tt][:, c0:c0 + cw])
        f_base += ng

    pool_w2.release()
    pool_h.release()
    pool_s4.release()
    pool_y.release()
    persist.release()
    psC.release()
    psD.release()


_cached_nc = None


def _get_nc():
    global _cached_nc
    if _cached_nc is None:
        _cached_nc = _build()
    return _cached_nc


def _host_masks(j):
    """Multiplicative masks: 1 where causal-valid, 0 where invalid.

    Slots 0..6 gate the main (post-AllGather) loop: ones for fully-valid
    super-chunks (s < p), zero otherwise — the diagonal super-chunk s == p is
    computed in the pre-pass from local k/v. Slot 7 holds the causal triangle
    used by that diagonal pre-pass (identical on every core)."""
    m = np.zeros((2, NSC, 2, LC, CH), np.float32)
    l_idx = np.arange(LC)[:, None]
    q_idx = np.arange(CH)[None, :]
    m1 = (l_idx <= q_idx).astype(np.float32)
    m2 = (l_idx + LC <= q_idx).astype(np.float32)
    for c in range(2):
        p = j if c == 0 else 7 - j
        for s in range(NSC - 1):
            if s < p:
                m[c, s, :, :, :] = 1.0
        m[c, 7, 0] = m1
        m[c, 7, 1] = m2
    return m


def _bf16(a):
    import jax.numpy as jnp
    return np.asarray(jnp.asarray(np.asarray(a, np.float32), jnp.bfloat16))


def kernel(x, g1, b1, w_qkv, bias_qkv, w_proj, bias_proj, g2, b2, w_fc,
           bias_fc, w_out, bias_out):
    x = np.asarray(x, np.float32)
    xf = x.reshape(B * S, D)

    # fold LN1 affine into qkv weights; pre-scale q by c^-0.5
    wqkv_m = (np.asarray(w_qkv) * np.asarray(g1)[:, None]).astype(np.float32)
    bqkv_m = (np.asarray(bias_qkv) + np.asarray(b1) @ np.asarray(w_qkv)).astype(
        np.float32)
    sc = 1.0 / np.sqrt(C)
    wqkv_m[:, :D] *= sc
    bqkv_m[:D] *= sc
    wfc_m = (np.asarray(w_fc) * np.asarray(g2)[:, None]).astype(np.float32)
    bfc_m = (np.asarray(bias_fc) + np.asarray(b2) @ np.asarray(w_fc)).astype(
        np.float32)

    common = {
        "wqkv": np.ascontiguousarray(_bf16(wqkv_m)),
        "bqkv": np.ascontiguousarray(bqkv_m),
        "wproj": np.ascontiguousarray(_bf16(w_proj)),
        "bproj": np.ascontiguousarray(np.asarray(bias_proj, np.float32)),
        "wfc": np.ascontiguousarray(_bf16(wfc_m)),
        "bfc": np.ascontiguousarray(bfc_m),
        "wout": np.ascontiguousarray(_bf16(w_out)),
        "bout": np.ascontiguousarray(np.asarray(bias_out, np.float32)),
    }
    in_maps = []
    for j in range(N_CORES):
        a0 = CH * j
        b0 = S + CH * (7 - j)
        xl = np.concatenate([xf[a0:a0 + CH], xf[b0:b0 + CH]], axis=0)
        in_maps.append({
            "x": np.ascontiguousarray(xl),
            "masks": np.ascontiguousarray(_bf16(_host_masks(j))),
            **common,
        })

    nc = _get_nc()
    res = run_bass_kernel_spmd(nc, in_maps, core_ids=list(range(N_CORES)))

    of = np.empty((B * S, D), np.float32)
    for j in range(N_CORES):
        o = res.results[j]["out"]
        a0 = CH * j
        b0 = S + CH * (7 - j)
        of[a0:a0 + CH] = o[:CH]
        of[b0:b0 + CH] = o[CH:]
    return of.reshape(B, S, D)
